# revision 14
# baseline (speedup 1.0000x reference)
"""Trainium2 Bass kernel for nn_ConstrainedEnhancementModel.

Contract: kernel(**inputs) takes the FULL unsharded inputs (as produced by
reference.setup_inputs()) and returns the FULL [4096, 2000, 6] float32 output.

Strategy (pure data parallel over 8 NeuronCores, 512 batch rows each):
  - Feature-major MLP chain: every hidden activation is stored [feat, batch]
    so torch-layout weights [fan_in, fan_out] are directly the matmul lhsT.
  - x is pre-arranged on the host into the window-blocked layout the kernel
    needs (no on-chip transposes), with the G-bias ones row baked in.
  - W6 (pre-scaled by the per-timestep blend coefficient) is stored fp8-e4m3,
    loaded over the SWDGE ring concurrently with the other loads, and kept
    fully resident in SBUF; the final layer runs DoubleRow fp8 matmuls
    (K=256 per instruction).
  - The constraint/interpolation epilogue is folded into the final matmul:
        out = h5 @ (W6 * c_dec) + x @ G + ones * (b6 * c_dec)
    where G is a sparse constant [600, 12000] matrix holding the linear
    interpolation + anchor/blend coefficients (bf16 path).
  - The output is written in bf16 (upcast to f32 on the host), halving the
    dominant HBM-write traffic; y DMAs alternate between the two HWDGE
    rings (SP / ACT) so neither ring's FIFO drain paces the main loop.
"""

import numpy as np
import ml_dtypes

import concourse.bass as bass
import concourse.bacc as bacc
import concourse.mybir as mybir
import concourse.tile as tile
from concourse import bass_utils

F32 = mybir.dt.float32
BF16 = mybir.dt.bfloat16
FP8 = mybir.dt.float8e4
BF16_NP = ml_dtypes.bfloat16
FP8_NP = ml_dtypes.float8_e4m3

# Problem config (hardcoded; must match the reference)
LOW_T = 100
HIGH_T = 2000
FEAT = 6
HID = 256
NUM_CLASSES = 10
LBL_DIM = 16
UP = 20
B = 4096
NCORES = 8
BC = B // NCORES          # 512 batch rows per core
NBT = BC // 128           # 4 batch tiles per core
D_IN = LOW_T * FEAT       # 600
D_OUT = HIGH_T * FEAT     # 12000
NW = 25                   # output windows (80 timesteps * 6 feats = 480 cols)
WT = 480
NI4 = 7                   # ceil(25/4) groups of 4 windows

DR = mybir.MatmulPerfMode.DoubleRow


def _build_nc():
    """Build the single-core Bass program (SPMD: same program on all 8)."""
    nc = bacc.Bacc("TRN2", target_bir_lowering=False, debug=False)

    xw_d = nc.dram_tensor("xw", [128, NI4 * 512], BF16, kind="ExternalInput")
    lab_d = nc.dram_tensor("labf", [1, BC], BF16, kind="ExternalInput")
    w1_d = nc.dram_tensor("w1re", [128, NI4 * 512], BF16, kind="ExternalInput")
    w2_d = nc.dram_tensor("w2", [128, 4 * 256], BF16, kind="ExternalInput")
    w3_d = nc.dram_tensor("w3", [128, 2 * 128], BF16, kind="ExternalInput")
    w4_d = nc.dram_tensor("w4", [128, 512], BF16, kind="ExternalInput")
    w5_d = nc.dram_tensor("w5", [128, 2 * 512], BF16, kind="ExternalInput")
    # window-pair-major W6: col block q = 4*window + 2*kp + j holds fp8
    # subtile (2*kp+j) of that window's 480 columns -> DoubleRow pairs sit
    # 480 B apart (small stride keeps the 2-per-cycle rhs fetch alive)
    w6_d = nc.dram_tensor("w6p8", [128, 4 * D_OUT], FP8, kind="ExternalInput")
    bia_d = nc.dram_tensor("bias", [128, 13], F32, kind="ExternalInput")
    emb_d = nc.dram_tensor("embT", [NUM_CLASSES, LBL_DIM], BF16, kind="ExternalInput")
    iota_d = nc.dram_tensor("iota10", [NUM_CLASSES, 1], F32, kind="ExternalInput")
    g_d = nc.dram_tensor("gmat", [128, NI4 * WT], BF16, kind="ExternalInput")
    y_d = nc.dram_tensor("y", [BC, D_OUT], BF16, kind="ExternalOutput")

    RELU = mybir.ActivationFunctionType.Relu

    with tile.TileContext(nc) as tc:
        with (
            tc.tile_pool(name="const", bufs=1) as cp,
            tc.tile_pool(name="outpool", bufs=8) as op,
            tc.tile_pool(name="ppool", bufs=8, space="PSUM") as pm,
        ):
            # ---- persistent SBUF tensors ----
            cw1 = cp.tile([128, NI4 * 512], BF16, tag="cw1", name="cw1")
            cw2 = cp.tile([128, 4 * 256], BF16, tag="cw2", name="cw2")
            cw3 = cp.tile([128, 2 * 128], BF16, tag="cw3", name="cw3")
            cw4 = cp.tile([128, 512], BF16, tag="cw4", name="cw4")
            cw5 = cp.tile([128, 2 * 512], BF16, tag="cw5", name="cw5")
            cw6 = cp.tile([128, 4 * NW, WT], FP8, tag="cw6", name="cw6")
            cb = cp.tile([128, 13], F32, tag="cb", name="cb")
            cemb = cp.tile([NUM_CLASSES, LBL_DIM], BF16, tag="cemb", name="cemb")
            ciota = cp.tile([NUM_CLASSES, 1], F32, tag="ciota", name="ciota")
            cg = cp.tile([128, NI4 * WT], BF16, tag="cg", name="cg")
            clab = cp.tile([1, BC], BF16, tag="clab", name="clab")
            ones10 = cp.tile([1, NUM_CLASSES], BF16, tag="ones10", name="ones10")
            xw = cp.tile([128, NI4 * 512], BF16, tag="xw", name="xw")
            h1 = [cp.tile([128, BC], BF16, tag=f"h1_{i}", name=f"h1_{i}") for i in range(4)]
            h2 = [cp.tile([128, BC], BF16, tag=f"h2_{i}", name=f"h2_{i}") for i in range(2)]
            feat = cp.tile([128, BC], BF16, tag="feat", name="feat")
            h4 = [cp.tile([128, BC], BF16, tag=f"h4_{i}", name=f"h4_{i}") for i in range(2)]
            h5 = cp.tile([128, 4, BC], FP8, tag="h5", name="h5")
            onehot = cp.tile([NUM_CLASSES, BC], BF16, tag="onehot", name="onehot")
            embt = cp.tile([LBL_DIM, BC], BF16, tag="embt", name="embt")

            # bias column layout in cb: b1 m0..3 | b2 m0..1 | b3 | b4 m0..1 | b5 m0..3
            B1, B2, B3, B4, B5 = 0, 4, 6, 7, 9

            # ---- const loads (SP ring; issue order = drain order) ----
            # xw/cw1 first: they gate L1, and every trigger costs ~700 ns of
            # SP sequencer time, so the encoder-critical loads go up front.
            nc.sync.dma_start(xw[:], xw_d[:])
            nc.sync.dma_start(cw1[:], w1_d[:])
            nc.sync.dma_start(clab[:], lab_d[:])
            nc.sync.dma_start(ciota[:], iota_d[:])
            nc.sync.dma_start(cemb[:], emb_d[:])
            nc.sync.dma_start(cw2[:], w2_d[:])
            nc.sync.dma_start(cw3[:], w3_d[:])
            nc.sync.dma_start(cw4[:], w4_d[:])
            nc.sync.dma_start(cw5[:], w5_d[:])
            nc.sync.dma_start(cb[:], bia_d[:])
            nc.sync.dma_start(cg[:], g_d[:])
            # W6 last: the SDMA engines shared-drain everything in flight, so
            # anything issued alongside W6 lands ~6 MB later; the encoder only
            # needs the loads above, and the final layer consumes W6 chunks
            # in issue order anyway.
            nc.gpsimd.memset(ones10[:], 1.0)
            for ks in range(4):
                nc.sync.dma_start(
                    cw6[:, ks * NW:(ks + 1) * NW, :],
                    w6_d[:, ks * D_OUT:(ks + 1) * D_OUT],
                )

            # ---- label one-hot + embedding (feature-major [16, BC]) ----
            psl = pm.tile([128, 512], F32, tag="ps", name="ps")
            nc.tensor.matmul(psl[0:NUM_CLASSES, 0:BC], ones10[:], clab[:],
                             start=True, stop=True)
            nc.vector.tensor_scalar(
                onehot[:], psl[0:NUM_CLASSES, 0:BC], ciota[:], None,
                mybir.AluOpType.is_equal,
            )
            pse = pm.tile([128, 512], F32, tag="ps", name="ps")
            nc.tensor.matmul(pse[0:LBL_DIM, 0:BC], cemb[:], onehot[:],
                             start=True, stop=True)
            nc.vector.tensor_copy(embt[:], pse[0:LBL_DIM, 0:BC])

            # ---- PE warm-up ----
            # The HAM clock gate holds the PE at 1.2 GHz until it sees ~3.4 us
            # of sustained activity.  While the encoder weights stream in,
            # burn dummy matmuls (only need clab/ones10, which land early) so
            # L1 starts at the full 2.4 GHz.
            for _ in range(14):
                psw = pm.tile([128, 512], F32, tag="ps", name="ps")
                nc.tensor.matmul(psw[0:NUM_CLASSES, :], ones10[:], clab[:],
                                 start=True, stop=True)

            # ---- encoder / decoder MLP (feature-major, N = BC) ----
            # L1: [600->512] via window-blocked x / rearranged W1
            for m in range(4):
                ps = pm.tile([128, 512], F32, tag="ps", name="ps")
                for i4 in range(NI4):
                    nc.tensor.matmul(
                        ps[:, 0:BC], cw1[:, i4 * 512 + m * 128:i4 * 512 + (m + 1) * 128],
                        xw[:, i4 * 512:(i4 + 1) * 512],
                        start=(i4 == 0), stop=(i4 == NI4 - 1),
                    )
                if m % 2 == 0:
                    nc.scalar.activation(h1[m][:], ps[:, 0:BC], RELU, bias=cb[:, B1 + m:B1 + m + 1])
                else:
                    nc.vector.tensor_scalar(h1[m][:], ps[:, 0:BC], cb[:, B1 + m:B1 + m + 1], 0.0, mybir.AluOpType.add, mybir.AluOpType.max)
            # L2: [512->256]
            for m in range(2):
                ps = pm.tile([128, 512], F32, tag="ps", name="ps")
                for k in range(4):
                    nc.tensor.matmul(
                        ps[:, 0:BC], cw2[:, k * 256 + m * 128:k * 256 + (m + 1) * 128], h1[k][:],
                        start=(k == 0), stop=(k == 3),
                    )
                if m % 2 == 0:
                    nc.scalar.activation(h2[m][:], ps[:, 0:BC], RELU, bias=cb[:, B2 + m:B2 + m + 1])
                else:
                    nc.vector.tensor_scalar(h2[m][:], ps[:, 0:BC], cb[:, B2 + m:B2 + m + 1], 0.0, mybir.AluOpType.add, mybir.AluOpType.max)
            # L3: [256->128], no relu
            ps = pm.tile([128, 512], F32, tag="ps", name="ps")
            for k in range(2):
                nc.tensor.matmul(ps[:, 0:BC], cw3[:, k * 128:(k + 1) * 128], h2[k][:],
                                 start=(k == 0), stop=(k == 1))
            nc.vector.tensor_scalar(feat[:], ps[:, 0:BC], cb[:, B3:B3 + 1], None, mybir.AluOpType.add)
            # L4: [144->256] = feat part + label-embedding part
            for m in range(2):
                ps = pm.tile([128, 512], F32, tag="ps", name="ps")
                nc.tensor.matmul(ps[:, 0:BC], cw4[:, m * 128:(m + 1) * 128],
                                 feat[:], start=True, stop=False)
                nc.tensor.matmul(ps[:, 0:BC], cw4[0:16, 256 + m * 128:256 + (m + 1) * 128],
                                 embt[:], start=False, stop=True)
                if m % 2 == 0:
                    nc.scalar.activation(h4[m][:], ps[:, 0:BC], RELU, bias=cb[:, B4 + m:B4 + m + 1])
                else:
                    nc.vector.tensor_scalar(h4[m][:], ps[:, 0:BC], cb[:, B4 + m:B4 + m + 1], 0.0, mybir.AluOpType.add, mybir.AluOpType.max)
            # L5: [256->512], output directly as fp8 k-subtiles of h5
            for m in range(4):
                ps = pm.tile([128, 512], F32, tag="ps", name="ps")
                for k in range(2):
                    nc.tensor.matmul(
                        ps[:, 0:BC], cw5[:, k * 512 + m * 128:k * 512 + (m + 1) * 128], h4[k][:],
                        start=(k == 0), stop=(k == 1),
                    )
                if m % 2 == 0:
                    nc.scalar.activation(h5[:, m, :], ps[:, 0:BC], RELU, bias=cb[:, B5 + m:B5 + m + 1])
                else:
                    nc.vector.tensor_scalar(h5[:, m, :], ps[:, 0:BC], cb[:, B5 + m:B5 + m + 1], 0.0, mybir.AluOpType.add, mybir.AluOpType.max)

            # ---- final layer + fused constraint epilogue ----
            # W6 fully SBUF-resident (fp8). Per (i4, bt): 4 windows get
            # 2 DoubleRow matmuls each (K=256 per instruction), then the four
            # K=32 G matmuls land on distinct PE row groups (concurrent),
            # then psum -> one [128, 1920] bf16 SBUF tile -> one y DMA,
            # alternating between the SP and ACT HWDGE rings.
            for i4 in range(NI4):
                nwin = 4 if i4 < 6 else 1
                for bt in range(NBT):
                    bsl = slice(bt * 128, (bt + 1) * 128)
                    pss = []
                    for w in range(nwin):
                        pss.append(pm.tile([128, 512], F32, tag="ps", name="ps")[:, 0:WT])
                    for kp in (0, 1):
                        for w in range(nwin):
                            i = 4 * i4 + w
                            nc.tensor.matmul(
                                pss[w][:], h5[:, 2 * kp:2 * kp + 2, bsl],
                                cw6[:, 4 * i + 2 * kp:4 * i + 2 * kp + 2, :],
                                start=(kp == 0), stop=False, perf_mode=DR,
                            )
                    for w in range(nwin):
                        p0 = 32 * w
                        nc.tensor.matmul(
                            pss[w][:],
                            xw[p0:p0 + 32, i4 * 512 + bt * 128:i4 * 512 + (bt + 1) * 128],
                            cg[p0:p0 + 32, i4 * WT:(i4 + 1) * WT],
                            start=False, stop=True, tile_position=(p0, 0),
                        )
                    ob = op.tile([128, 4 * WT], BF16, tag="ob", name="ob")
                    for w in range(nwin):
                        if w % 2 == 0:
                            nc.vector.tensor_copy(ob[:, w * WT:(w + 1) * WT], pss[w][:])
                        else:
                            nc.scalar.copy(ob[:, w * WT:(w + 1) * WT], pss[w][:])
                    eng = nc.sync if (i4 * NBT + bt) % 2 == 0 else nc.scalar
                    eng.dma_start(
                        y_d[bsl, i4 * 4 * WT:i4 * 4 * WT + nwin * WT],
                        ob[:, 0:nwin * WT],
                    )

    nc.compile()
    return nc


def _host_prep(inputs):
    """Build per-core in_maps from the full inputs."""
    x_full = np.asarray(inputs["low_res_data"], np.float32).reshape(B, D_IN)
    labels = np.asarray(inputs["labels"]).astype(np.float32)
    W1 = np.asarray(inputs["W1"], np.float32)
    W6 = np.asarray(inputs["W6"], np.float32)
    b6 = np.asarray(inputs["b6"], np.float32)

    # per-timestep blend coefficients (match the reference formulas)
    t = np.arange(HIGH_T)
    seg = np.clip(t // UP, 0, LOW_T - 2)
    alpha = ((t - seg * UP) / UP).astype(np.float64)
    is_anchor = (t % UP) == 0
    interior = t < (LOW_T - 1) * UP
    blendf = np.where(is_anchor, 1.0, np.where(interior, 0.8, 0.0))
    c_d = np.where(is_anchor, 0.0, np.where(interior, 0.2, 1.0))
    c_start = blendf * (1.0 - alpha)
    c_end = blendf * alpha

    # G matrix, window-blocked: [128, NI4*480]; window i lives at partition
    # offset 32*(i%4), col block i//4.  Rows r=0..29 <-> x col 24*i + r,
    # row 30 = bias row (paired with the constant-1.0 row of xw).
    gmat = np.zeros((128, NI4 * WT), np.float64)
    for tt in range(HIGH_T):
        i, dt = divmod(tt, 80)
        i4, wpos = divmod(i, 4)
        p0 = 32 * wpos
        sl = seg[tt] - 4 * i
        for f in range(FEAT):
            col = i4 * WT + FEAT * dt + f
            gmat[p0 + FEAT * sl + f, col] += c_start[tt]
            gmat[p0 + FEAT * (sl + 1) + f, col] += c_end[tt]
            gmat[p0 + 30, col] = c_d[tt] * np.float64(b6[FEAT * tt + f])
    gmat = gmat.astype(np.float32).astype(BF16_NP)

    c_d_full = np.repeat(c_d, FEAT).astype(np.float32)
    # window-pair-major fp8 W6: [s=subtile, p, i=window, c] -> [p, i, s, c]
    w6p = (
        (W6 * c_d_full[None, :]).astype(FP8_NP)
        .reshape(4, 128, NW, WT).transpose(1, 2, 0, 3).reshape(128, 4 * D_OUT)
        .copy()
    )

    # W1 rearranged to the window-blocked x layout (duplicated/ones/pad rows
    # get zero weights); flattened [128, NI4*512] with i4 blocks side by side
    w1re = np.zeros((128, NI4 * 512), np.float32)
    for c in range(D_IN):
        i, r = divmod(c, 24)
        i4, wpos = divmod(i, 4)
        w1re[32 * wpos + r, i4 * 512:(i4 + 1) * 512] = W1[c, :]
    w1re = w1re.astype(BF16_NP)

    w4 = np.zeros((128, 512), np.float32)
    w4[:, 0:256] = np.asarray(inputs["W4"], np.float32)[:128]
    w4[0:16, 256:512] = np.asarray(inputs["W4"], np.float32)[128:144]

    bias = np.zeros((128, 13), np.float32)
    bias[:, 0:4] = np.asarray(inputs["b1"], np.float32).reshape(4, 128).T
    bias[:, 4:6] = np.asarray(inputs["b2"], np.float32).reshape(2, 128).T
    bias[:, 6] = np.asarray(inputs["b3"], np.float32)
    bias[:, 7:9] = np.asarray(inputs["b4"], np.float32).reshape(2, 128).T
    bias[:, 9:13] = np.asarray(inputs["b5"], np.float32).reshape(4, 128).T

    const_map = {
        "w1re": w1re,
        "w2": np.asarray(inputs["W2"], np.float32).reshape(4, 128, 256).transpose(1, 0, 2).reshape(128, 1024).copy().astype(BF16_NP),
        "w3": np.asarray(inputs["W3"], np.float32).reshape(2, 128, 128).transpose(1, 0, 2).reshape(128, 256).copy().astype(BF16_NP),
        "w4": w4.astype(BF16_NP),
        "w5": np.asarray(inputs["W5"], np.float32).reshape(2, 128, 512).transpose(1, 0, 2).reshape(128, 1024).copy().astype(BF16_NP),
        "w6p8": w6p,
        "bias": bias,
        "embT": np.asarray(inputs["emb"], np.float32).astype(BF16_NP),
        "iota10": np.arange(NUM_CLASSES, dtype=np.float32).reshape(NUM_CLASSES, 1),
        "gmat": gmat,
    }

    # window-blocked x layout: [128, NI4*512]; window i = 4*i4 + wpos:
    # partition 32*wpos + r (r<30) = x col 24*i + r; row 30 = 1.0 (G bias);
    # row 31 = 0.  Column = i4*512 + batch row within the core chunk.
    in_maps = []
    for c in range(NCORES):
        sl = slice(c * BC, (c + 1) * BC)
        xc = x_full[sl]                                    # [BC, 600]
        xwin = np.zeros((128, NI4 * 512), np.float32)
        for i4 in range(NI4):
            nwin = 4 if i4 < 6 else 1
            blk = xwin[:, i4 * 512:(i4 + 1) * 512]
            for wpos in range(nwin):
                i = 4 * i4 + wpos
                c0 = 24 * i
                ncols = min(30, D_IN - c0)
                blk[32 * wpos:32 * wpos + ncols, :] = xc[:, c0:c0 + ncols].T
                blk[32 * wpos + 30, :] = 1.0
        m = dict(const_map)
        m["xw"] = xwin.astype(BF16_NP)
        m["labf"] = labels[sl].reshape(1, BC).astype(BF16_NP)
        in_maps.append(m)
    return in_maps


_NC_CACHE = None


def kernel(**inputs) -> np.ndarray:
    global _NC_CACHE
    if _NC_CACHE is None:
        _NC_CACHE = _build_nc()
    nc = _NC_CACHE
    in_maps = _host_prep(inputs)
    res = bass_utils.run_bass_kernel_spmd(nc, in_maps, core_ids=list(range(NCORES)))
    out = np.concatenate(
        [np.asarray(res.results[c]["y"]).astype(np.float32) for c in range(NCORES)],
        axis=0,
    )
    return out.reshape(B, HIGH_T, FEAT)


# revision 16
# speedup vs baseline: 1.2144x; 1.2144x over previous
"""Trainium2 Bass kernel for nn_ConstrainedEnhancementModel.

Contract: kernel(**inputs) takes the FULL unsharded inputs (as produced by
reference.setup_inputs()) and returns the FULL [4096, 2000, 6] float32 output.

Strategy (pure data parallel over 8 NeuronCores, 512 batch rows each):
  - Feature-major MLP chain: every hidden activation is stored [feat, batch]
    so torch-layout weights [fan_in, fan_out] are directly the matmul lhsT.
  - x is pre-arranged on the host into the window-blocked layout the kernel
    needs (no on-chip transposes), with the G-bias ones row baked in.
  - W6 (pre-scaled by the per-timestep blend coefficient) is stored fp8-e4m3,
    loaded over the SWDGE ring concurrently with the other loads, and kept
    fully resident in SBUF; the final layer runs DoubleRow fp8 matmuls
    (K=256 per instruction).
  - The constraint/interpolation epilogue is folded into the final matmul:
        out = h5 @ (W6 * c_dec) + x @ G + ones * (b6 * c_dec)
    where G is a sparse constant [600, 12000] matrix holding the linear
    interpolation + anchor/blend coefficients (bf16 path).
  - The output is written in bf16 (upcast to f32 on the host), halving the
    dominant HBM-write traffic; y DMAs alternate between the two HWDGE
    rings (SP / ACT) so neither ring's FIFO drain paces the main loop.
"""

import numpy as np
import ml_dtypes

import concourse.bass as bass
import concourse.bacc as bacc
import concourse.mybir as mybir
import concourse.tile as tile
from concourse import bass_utils

F32 = mybir.dt.float32
BF16 = mybir.dt.bfloat16
FP8 = mybir.dt.float8e4
BF16_NP = ml_dtypes.bfloat16
FP8_NP = ml_dtypes.float8_e4m3

# Problem config (hardcoded; must match the reference)
LOW_T = 100
HIGH_T = 2000
FEAT = 6
HID = 256
NUM_CLASSES = 10
LBL_DIM = 16
UP = 20
B = 4096
NCORES = 8
BC = B // NCORES          # 512 batch rows per core
NBT = BC // 128           # 4 batch tiles per core
D_IN = LOW_T * FEAT       # 600
D_OUT = HIGH_T * FEAT     # 12000
NW = 25                   # output windows (80 timesteps * 6 feats = 480 cols)
WT = 480
NI4 = 7                   # ceil(25/4) groups of 4 windows

DR = mybir.MatmulPerfMode.DoubleRow


def _build_nc():
    """Build the single-core Bass program (SPMD: same program on all 8)."""
    nc = bacc.Bacc("TRN2", target_bir_lowering=False, debug=False)

    xw_d = nc.dram_tensor("xw", [128, NI4 * 512], BF16, kind="ExternalInput")
    lab_d = nc.dram_tensor("labf", [1, BC], BF16, kind="ExternalInput")
    w1_d = nc.dram_tensor("w1re", [128, NI4 * 512], BF16, kind="ExternalInput")
    w2_d = nc.dram_tensor("w2", [128, 4 * 256], BF16, kind="ExternalInput")
    w3_d = nc.dram_tensor("w3", [128, 2 * 128], BF16, kind="ExternalInput")
    w4_d = nc.dram_tensor("w4", [128, 512], BF16, kind="ExternalInput")
    w5_d = nc.dram_tensor("w5", [128, 2 * 512], BF16, kind="ExternalInput")
    # window-pair-major W6: col block q = 4*window + 2*kp + j holds fp8
    # subtile (2*kp+j) of that window's 480 columns -> DoubleRow pairs sit
    # 480 B apart (small stride keeps the 2-per-cycle rhs fetch alive)
    w6_d = nc.dram_tensor("w6p8", [128, 4 * D_OUT], FP8, kind="ExternalInput")
    bia_d = nc.dram_tensor("bias", [128, 13], F32, kind="ExternalInput")
    emb_d = nc.dram_tensor("embT", [NUM_CLASSES, LBL_DIM], BF16, kind="ExternalInput")
    iota_d = nc.dram_tensor("iota10", [NUM_CLASSES, 1], F32, kind="ExternalInput")
    g_d = nc.dram_tensor("gmat", [128, NI4 * WT], BF16, kind="ExternalInput")
    y_d = nc.dram_tensor("y", [BC, D_OUT], BF16, kind="ExternalOutput")

    RELU = mybir.ActivationFunctionType.Relu

    with tile.TileContext(nc) as tc:
        with (
            tc.tile_pool(name="const", bufs=1) as cp,
            tc.tile_pool(name="outpool", bufs=8) as op,
            tc.tile_pool(name="ppool", bufs=8, space="PSUM") as pm,
        ):
            # ---- persistent SBUF tensors ----
            cw1 = cp.tile([128, NI4 * 512], BF16, tag="cw1", name="cw1")
            cw2 = cp.tile([128, 4 * 256], BF16, tag="cw2", name="cw2")
            cw3 = cp.tile([128, 2 * 128], BF16, tag="cw3", name="cw3")
            cw4 = cp.tile([128, 512], BF16, tag="cw4", name="cw4")
            cw5 = cp.tile([128, 2 * 512], BF16, tag="cw5", name="cw5")
            cw6 = cp.tile([128, 4 * NW, WT], FP8, tag="cw6", name="cw6")
            cb = cp.tile([128, 13], F32, tag="cb", name="cb")
            cemb = cp.tile([NUM_CLASSES, LBL_DIM], BF16, tag="cemb", name="cemb")
            ciota = cp.tile([NUM_CLASSES, 1], F32, tag="ciota", name="ciota")
            cg = cp.tile([128, NI4 * WT], BF16, tag="cg", name="cg")
            clab = cp.tile([1, BC], BF16, tag="clab", name="clab")
            ones10 = cp.tile([1, NUM_CLASSES], BF16, tag="ones10", name="ones10")
            xw = cp.tile([128, NI4 * 512], BF16, tag="xw", name="xw")
            h1 = [cp.tile([128, BC], BF16, tag=f"h1_{i}", name=f"h1_{i}") for i in range(4)]
            h2 = [cp.tile([128, BC], BF16, tag=f"h2_{i}", name=f"h2_{i}") for i in range(2)]
            feat = cp.tile([128, BC], BF16, tag="feat", name="feat")
            h4 = [cp.tile([128, BC], BF16, tag=f"h4_{i}", name=f"h4_{i}") for i in range(2)]
            h5 = cp.tile([128, 4, BC], FP8, tag="h5", name="h5")
            onehot = cp.tile([NUM_CLASSES, BC], BF16, tag="onehot", name="onehot")
            embt = cp.tile([LBL_DIM, BC], BF16, tag="embt", name="embt")

            # bias column layout in cb: b1 m0..3 | b2 m0..1 | b3 | b4 m0..1 | b5 m0..3
            B1, B2, B3, B4, B5 = 0, 4, 6, 7, 9

            # ---- const loads (SP ring; issue order = drain order) ----
            # tiny label-path loads first (their data lands in ~1 us and the
            # label matmuls + PE warm-up only need these), then the
            # encoder-critical xw/cw1, then the rest, then W6.
            nc.sync.dma_start(clab[:], lab_d[:])
            nc.sync.dma_start(ciota[:], iota_d[:])
            nc.sync.dma_start(cemb[:], emb_d[:])
            nc.sync.dma_start(xw[:], xw_d[:])
            nc.sync.dma_start(cw1[:], w1_d[:])
            nc.sync.dma_start(cw2[:], w2_d[:])
            nc.sync.dma_start(cw3[:], w3_d[:])
            nc.sync.dma_start(cw4[:], w4_d[:])
            nc.sync.dma_start(cw5[:], w5_d[:])
            nc.sync.dma_start(cb[:], bia_d[:])
            nc.sync.dma_start(cg[:], g_d[:])
            # W6 last: the SDMA engines shared-drain everything in flight, so
            # anything issued alongside W6 lands ~6 MB later; the encoder only
            # needs the loads above, and the final layer consumes W6 chunks
            # in issue order anyway.
            nc.gpsimd.memset(ones10[:], 1.0)
            for ks in range(4):
                nc.sync.dma_start(
                    cw6[:, ks * NW:(ks + 1) * NW, :],
                    w6_d[:, ks * D_OUT:(ks + 1) * D_OUT],
                )

            # ---- label one-hot + embedding (feature-major [16, BC]) ----
            psl = pm.tile([128, 512], F32, tag="ps", name="ps")
            nc.tensor.matmul(psl[0:NUM_CLASSES, 0:BC], ones10[:], clab[:],
                             start=True, stop=True)
            nc.vector.tensor_scalar(
                onehot[:], psl[0:NUM_CLASSES, 0:BC], ciota[:], None,
                mybir.AluOpType.is_equal,
            )
            pse = pm.tile([128, 512], F32, tag="ps", name="ps")
            nc.tensor.matmul(pse[0:LBL_DIM, 0:BC], cemb[:], onehot[:],
                             start=True, stop=True)
            nc.vector.tensor_copy(embt[:], pse[0:LBL_DIM, 0:BC])

            # ---- PE warm-up ----
            # The HAM clock gate holds the PE at 1.2 GHz until it sees ~3.4 us
            # of sustained activity.  While the encoder weights stream in,
            # burn dummy matmuls (only need clab/ones10, which land early) so
            # L1 starts at the full 2.4 GHz.
            for _ in range(14):
                psw = pm.tile([128, 512], F32, tag="ps", name="ps")
                nc.tensor.matmul(psw[0:NUM_CLASSES, :], ones10[:], clab[:],
                                 start=True, stop=True)

            # ---- encoder / decoder MLP (feature-major, N = BC) ----
            # L1: [600->512] via window-blocked x / rearranged W1
            for m in range(4):
                ps = pm.tile([128, 512], F32, tag="ps", name="ps")
                for i4 in range(NI4):
                    nc.tensor.matmul(
                        ps[:, 0:BC], cw1[:, i4 * 512 + m * 128:i4 * 512 + (m + 1) * 128],
                        xw[:, i4 * 512:(i4 + 1) * 512],
                        start=(i4 == 0), stop=(i4 == NI4 - 1),
                    )
                if m % 2 == 0:
                    nc.scalar.activation(h1[m][:], ps[:, 0:BC], RELU, bias=cb[:, B1 + m:B1 + m + 1])
                else:
                    nc.vector.tensor_scalar(h1[m][:], ps[:, 0:BC], cb[:, B1 + m:B1 + m + 1], 0.0, mybir.AluOpType.add, mybir.AluOpType.max)
            # L2: [512->256]
            for m in range(2):
                ps = pm.tile([128, 512], F32, tag="ps", name="ps")
                for k in range(4):
                    nc.tensor.matmul(
                        ps[:, 0:BC], cw2[:, k * 256 + m * 128:k * 256 + (m + 1) * 128], h1[k][:],
                        start=(k == 0), stop=(k == 3),
                    )
                if m % 2 == 0:
                    nc.scalar.activation(h2[m][:], ps[:, 0:BC], RELU, bias=cb[:, B2 + m:B2 + m + 1])
                else:
                    nc.vector.tensor_scalar(h2[m][:], ps[:, 0:BC], cb[:, B2 + m:B2 + m + 1], 0.0, mybir.AluOpType.add, mybir.AluOpType.max)
            # L3: [256->128], no relu
            ps = pm.tile([128, 512], F32, tag="ps", name="ps")
            for k in range(2):
                nc.tensor.matmul(ps[:, 0:BC], cw3[:, k * 128:(k + 1) * 128], h2[k][:],
                                 start=(k == 0), stop=(k == 1))
            nc.vector.tensor_scalar(feat[:], ps[:, 0:BC], cb[:, B3:B3 + 1], None, mybir.AluOpType.add)
            # L4: [144->256] = feat part + label-embedding part
            for m in range(2):
                ps = pm.tile([128, 512], F32, tag="ps", name="ps")
                nc.tensor.matmul(ps[:, 0:BC], cw4[:, m * 128:(m + 1) * 128],
                                 feat[:], start=True, stop=False)
                nc.tensor.matmul(ps[:, 0:BC], cw4[0:16, 256 + m * 128:256 + (m + 1) * 128],
                                 embt[:], start=False, stop=True)
                if m % 2 == 0:
                    nc.scalar.activation(h4[m][:], ps[:, 0:BC], RELU, bias=cb[:, B4 + m:B4 + m + 1])
                else:
                    nc.vector.tensor_scalar(h4[m][:], ps[:, 0:BC], cb[:, B4 + m:B4 + m + 1], 0.0, mybir.AluOpType.add, mybir.AluOpType.max)
            # L5: [256->512], output directly as fp8 k-subtiles of h5
            for m in range(4):
                ps = pm.tile([128, 512], F32, tag="ps", name="ps")
                for k in range(2):
                    nc.tensor.matmul(
                        ps[:, 0:BC], cw5[:, k * 512 + m * 128:k * 512 + (m + 1) * 128], h4[k][:],
                        start=(k == 0), stop=(k == 1),
                    )
                if m % 2 == 0:
                    nc.scalar.activation(h5[:, m, :], ps[:, 0:BC], RELU, bias=cb[:, B5 + m:B5 + m + 1])
                else:
                    nc.vector.tensor_scalar(h5[:, m, :], ps[:, 0:BC], cb[:, B5 + m:B5 + m + 1], 0.0, mybir.AluOpType.add, mybir.AluOpType.max)

            # ---- final layer + fused constraint epilogue ----
            # W6 fully SBUF-resident (fp8). Per (i4, bt): 4 windows get
            # 2 DoubleRow matmuls each (K=256 per instruction), then the four
            # K=32 G matmuls land on distinct PE row groups (concurrent),
            # then psum -> one [128, 1920] bf16 SBUF tile -> one y DMA,
            # alternating between the SP and ACT HWDGE rings.
            for i4 in range(NI4):
                nwin = 4 if i4 < 6 else 1
                for bt in range(NBT):
                    bsl = slice(bt * 128, (bt + 1) * 128)
                    pss = []
                    for w in range(nwin):
                        pss.append(pm.tile([128, 512], F32, tag="ps", name="ps")[:, 0:WT])
                    for kp in (0, 1):
                        for w in range(nwin):
                            i = 4 * i4 + w
                            nc.tensor.matmul(
                                pss[w][:], h5[:, 2 * kp:2 * kp + 2, bsl],
                                cw6[:, 4 * i + 2 * kp:4 * i + 2 * kp + 2, :],
                                start=(kp == 0), stop=False, perf_mode=DR,
                            )
                    for w in range(nwin):
                        p0 = 32 * w
                        nc.tensor.matmul(
                            pss[w][:],
                            xw[p0:p0 + 32, i4 * 512 + bt * 128:i4 * 512 + (bt + 1) * 128],
                            cg[p0:p0 + 32, i4 * WT:(i4 + 1) * WT],
                            start=False, stop=True, tile_position=(p0, 0),
                        )
                    ob = op.tile([128, 4 * WT], BF16, tag="ob", name="ob")
                    for w in range(nwin):
                        if w % 2 == 0:
                            nc.vector.tensor_copy(ob[:, w * WT:(w + 1) * WT], pss[w][:])
                        else:
                            nc.scalar.copy(ob[:, w * WT:(w + 1) * WT], pss[w][:])
                    nc.sync.dma_start(
                        y_d[bsl, i4 * 4 * WT:i4 * 4 * WT + nwin * WT],
                        ob[:, 0:nwin * WT],
                    )

    nc.compile()
    return nc


def _host_prep(inputs):
    """Build per-core in_maps from the full inputs."""
    x_full = np.asarray(inputs["low_res_data"], np.float32).reshape(B, D_IN)
    labels = np.asarray(inputs["labels"]).astype(np.float32)
    W1 = np.asarray(inputs["W1"], np.float32)
    W6 = np.asarray(inputs["W6"], np.float32)
    b6 = np.asarray(inputs["b6"], np.float32)

    # per-timestep blend coefficients (match the reference formulas)
    t = np.arange(HIGH_T)
    seg = np.clip(t // UP, 0, LOW_T - 2)
    alpha = ((t - seg * UP) / UP).astype(np.float64)
    is_anchor = (t % UP) == 0
    interior = t < (LOW_T - 1) * UP
    blendf = np.where(is_anchor, 1.0, np.where(interior, 0.8, 0.0))
    c_d = np.where(is_anchor, 0.0, np.where(interior, 0.2, 1.0))
    c_start = blendf * (1.0 - alpha)
    c_end = blendf * alpha

    # G matrix, window-blocked: [128, NI4*480]; window i lives at partition
    # offset 32*(i%4), col block i//4.  Rows r=0..29 <-> x col 24*i + r,
    # row 30 = bias row (paired with the constant-1.0 row of xw).
    gmat = np.zeros((128, NI4 * WT), np.float64)
    for tt in range(HIGH_T):
        i, dt = divmod(tt, 80)
        i4, wpos = divmod(i, 4)
        p0 = 32 * wpos
        sl = seg[tt] - 4 * i
        for f in range(FEAT):
            col = i4 * WT + FEAT * dt + f
            gmat[p0 + FEAT * sl + f, col] += c_start[tt]
            gmat[p0 + FEAT * (sl + 1) + f, col] += c_end[tt]
            gmat[p0 + 30, col] = c_d[tt] * np.float64(b6[FEAT * tt + f])
    gmat = gmat.astype(np.float32).astype(BF16_NP)

    c_d_full = np.repeat(c_d, FEAT).astype(np.float32)
    # window-pair-major fp8 W6: [s=subtile, p, i=window, c] -> [p, i, s, c]
    w6p = (
        (W6 * c_d_full[None, :]).astype(FP8_NP)
        .reshape(4, 128, NW, WT).transpose(1, 2, 0, 3).reshape(128, 4 * D_OUT)
        .copy()
    )

    # W1 rearranged to the window-blocked x layout (duplicated/ones/pad rows
    # get zero weights); flattened [128, NI4*512] with i4 blocks side by side
    w1re = np.zeros((128, NI4 * 512), np.float32)
    for c in range(D_IN):
        i, r = divmod(c, 24)
        i4, wpos = divmod(i, 4)
        w1re[32 * wpos + r, i4 * 512:(i4 + 1) * 512] = W1[c, :]
    w1re = w1re.astype(BF16_NP)

    w4 = np.zeros((128, 512), np.float32)
    w4[:, 0:256] = np.asarray(inputs["W4"], np.float32)[:128]
    w4[0:16, 256:512] = np.asarray(inputs["W4"], np.float32)[128:144]

    bias = np.zeros((128, 13), np.float32)
    bias[:, 0:4] = np.asarray(inputs["b1"], np.float32).reshape(4, 128).T
    bias[:, 4:6] = np.asarray(inputs["b2"], np.float32).reshape(2, 128).T
    bias[:, 6] = np.asarray(inputs["b3"], np.float32)
    bias[:, 7:9] = np.asarray(inputs["b4"], np.float32).reshape(2, 128).T
    bias[:, 9:13] = np.asarray(inputs["b5"], np.float32).reshape(4, 128).T

    const_map = {
        "w1re": w1re,
        "w2": np.asarray(inputs["W2"], np.float32).reshape(4, 128, 256).transpose(1, 0, 2).reshape(128, 1024).copy().astype(BF16_NP),
        "w3": np.asarray(inputs["W3"], np.float32).reshape(2, 128, 128).transpose(1, 0, 2).reshape(128, 256).copy().astype(BF16_NP),
        "w4": w4.astype(BF16_NP),
        "w5": np.asarray(inputs["W5"], np.float32).reshape(2, 128, 512).transpose(1, 0, 2).reshape(128, 1024).copy().astype(BF16_NP),
        "w6p8": w6p,
        "bias": bias,
        "embT": np.asarray(inputs["emb"], np.float32).astype(BF16_NP),
        "iota10": np.arange(NUM_CLASSES, dtype=np.float32).reshape(NUM_CLASSES, 1),
        "gmat": gmat,
    }

    # window-blocked x layout: [128, NI4*512]; window i = 4*i4 + wpos:
    # partition 32*wpos + r (r<30) = x col 24*i + r; row 30 = 1.0 (G bias);
    # row 31 = 0.  Column = i4*512 + batch row within the core chunk.
    in_maps = []
    for c in range(NCORES):
        sl = slice(c * BC, (c + 1) * BC)
        xc = x_full[sl]                                    # [BC, 600]
        xwin = np.zeros((128, NI4 * 512), np.float32)
        for i4 in range(NI4):
            nwin = 4 if i4 < 6 else 1
            blk = xwin[:, i4 * 512:(i4 + 1) * 512]
            for wpos in range(nwin):
                i = 4 * i4 + wpos
                c0 = 24 * i
                ncols = min(30, D_IN - c0)
                blk[32 * wpos:32 * wpos + ncols, :] = xc[:, c0:c0 + ncols].T
                blk[32 * wpos + 30, :] = 1.0
        m = dict(const_map)
        m["xw"] = xwin.astype(BF16_NP)
        m["labf"] = labels[sl].reshape(1, BC).astype(BF16_NP)
        in_maps.append(m)
    return in_maps


_NC_CACHE = None


def kernel(**inputs) -> np.ndarray:
    global _NC_CACHE
    if _NC_CACHE is None:
        _NC_CACHE = _build_nc()
    nc = _NC_CACHE
    in_maps = _host_prep(inputs)
    res = bass_utils.run_bass_kernel_spmd(nc, in_maps, core_ids=list(range(NCORES)))
    out = np.concatenate(
        [np.asarray(res.results[c]["y"]).astype(np.float32) for c in range(NCORES)],
        axis=0,
    )
    return out.reshape(B, HIGH_T, FEAT)
